# revision 44
# baseline (speedup 1.0000x reference)
"""Trainium2 Bass kernel for nn_PredictAverageReward.

Per core (fruits sharded 8 ways, 512 fruits each):
  1. fp32 GEMM chain on TensorE produces Rd [512 fruits, 256 tools] laid out
     as one SBUF tile rd_all [128, 4*256] (4 fruit blocks along free axis).
     min_r is dropped: it cancels in all comparisons and the output depends
     only on comparisons.
  2. Pairwise-win counting. For each column j we need
     ge[f, i] = (Rd[f, i] >= Rd[f, j]) for i < j, then counts = sum_f ge.
     Compares are split across three engines:
       - VectorE merged tensor_tensor with a stride-0 broadcast comparand
         (one instruction covers all 4 fruit blocks) for small/mid j,
       - VectorE tensor_scalar         (4 instrs/j) for mid j,
       - GpSimd  tensor_scalar         (4 instrs/j) for a mid slice,
       - ScalarE Sign activation       (4 instrs/j, values in {-1,0,1}) for
         large j.
     ge is written as bf16 (exact for 0/±1) so the reduction matmuls skip the
     fp32 LOW/HIGH split. Reduction: one bf16 matmul per (j, block) whose
     stationary operand is a [128, 32] one-hot column routing the sums into
     PSUM row j%128 of col-group (j%128)//32; all rows accumulate into two
     pre-zeroed PSUM banks (all start=False), drained with two copies.
  3. Host: sum per-core counts, threshold at 2048, build the proposal.
"""

import sys

for _p in ("/opt/trn_rl_repo",):
    if _p not in sys.path:
        sys.path.insert(0, _p)

import numpy as np

import concourse.bass as bass
import concourse.bacc as bacc
import concourse.mybir as mybir
import concourse.tile as tile
from concourse.bass_utils import run_bass_kernel_spmd

F32 = mybir.dt.float32
BF16 = mybir.dt.bfloat16

N_CORES = 8
N_FRUITS, N_TOOLS, P_F, P_T, D = 8192, 1024, 128, 128, 512
K_DOMAIN, BATCH = 256, 4096
F_PER_CORE = BATCH // N_CORES          # 512 fruits per core
N_FT = F_PER_CORE // 128               # 4 fruit blocks

# engine split over j (tunable):
T_GPS_LO, T_GPS_HI = 0, 0       # [lo, hi): GpSimd tensor_scalar (off: too slow)
T_ACT = 206                     # j >= T_ACT: ScalarE Sign  (sign-sum rows)
T_TT = 206                      # j < T_TT: DVE merged tensor_tensor

TRACE = False
LAST_RESULTS = None


def _j_engine(j):
    """-> 'tt' (DVE merged), 'ts' (DVE tensor_scalar), 'gps', 'act'"""
    if T_GPS_LO <= j < T_GPS_HI:
        return "gps"
    if j >= T_ACT:
        return "act"
    if j < T_TT:
        return "tt"
    return "ts"


def _sign_rows():
    return set(j for j in range(1, 256) if _j_engine(j) == "act")


def _build_nc():
    nc = bacc.Bacc()
    K = K_DOMAIN

    g_t = nc.dram_tensor("g_t", [128, F_PER_CORE], F32, kind="ExternalInput")
    mf = nc.dram_tensor("mf", [P_F, D], F32, kind="ExternalInput")
    wt_in = nc.dram_tensor("wt_in", [D, K], F32, kind="ExternalInput")
    counts = nc.dram_tensor("counts", [128, 2 * K], F32, kind="ExternalOutput")

    with tile.TileContext(nc) as tc:
        with (
            tc.tile_pool(name="persist", bufs=1) as pp,
            tc.tile_pool(name="ge", bufs=8) as gep,
            tc.tile_pool(name="ge1", bufs=48) as gact_pool,
            tc.tile_pool(name="gemm_ps", bufs=3, space=bass.MemorySpace.PSUM) as gps_pool,
            tc.tile_pool(name="cnt_ps", bufs=2, space=bass.MemorySpace.PSUM) as cps,
        ):
            # ---- constants ----
            # strip[:, 31] = 1 else 0; strip[:, 31-m:63-m] is a [128, 32]
            # one-hot-column matrix selecting row m of a 32-wide col-group.
            strip = pp.tile([128, 63], BF16, tag="strip")
            nc.gpsimd.memset(strip[:], 0.0)
            nc.gpsimd.memset(strip[:, 31:32], 1.0)
            zw = pp.tile([128, 128], BF16, tag="zw")
            nc.gpsimd.memset(zw[:], 0.0)
            zsb = pp.tile([128, K], BF16, tag="zsb")
            nc.gpsimd.memset(zsb[:], 0.0)

            # ---- load inputs ----
            gt_sb = pp.tile([128, F_PER_CORE], F32, tag="gt")
            nc.sync.dma_start(gt_sb[:], g_t[:])
            mf_sb = pp.tile([128, D], F32, tag="mf")
            nc.sync.dma_start(mf_sb[:], mf[:])
            # WT = ((tools_prop[domain_t] @ M_tool) @ M).T precomputed on host
            wt_sb = []
            for dt in range(4):
                t = pp.tile([128, K], F32, tag=f"wt{dt}")
                nc.sync.dma_start(t[:], wt_in[dt * 128:(dt + 1) * 128, :])
                wt_sb.append(t)

            # ---- fp32 GEMM chain ----
            pmfT_sb = []
            for dt in range(4):
                ps = gps_pool.tile([128, F_PER_CORE], F32)
                nc.tensor.matmul(ps[:], mf_sb[:, dt * 128:(dt + 1) * 128],
                                 gt_sb[:], start=True, stop=True)
                t = pp.tile([128, F_PER_CORE], F32, tag=f"pmfT{dt}")
                if dt % 2 == 0:
                    nc.scalar.copy(t[:], ps[:])
                else:
                    nc.vector.tensor_copy(t[:], ps[:])
                pmfT_sb.append(t)

            # Rd blocks stay resident in PSUM: ScalarE compares read PSUM
            # (cheaper init than SBUF); DVE compares read the SBUF copy.
            rd_all = pp.tile([128, N_FT * K], F32, tag="rd")
            neg_all = pp.tile([128, N_FT * K], F32, tag="neg")
            for ft in range(N_FT):
                ps = gps_pool.tile([128, K], F32)
                for dt in range(4):
                    nc.tensor.matmul(ps[:], pmfT_sb[dt][:, ft * 128:(ft + 1) * 128],
                                     wt_sb[dt][:], start=(dt == 0), stop=(dt == 3))
                nc.vector.tensor_copy(rd_all[:, ft * K:(ft + 1) * K], ps[:])
                if ft % 2 == 0:
                    nc.scalar.mul(neg_all[:, ft * K:(ft + 1) * K], ps[:], -1.0)
                else:
                    nc.vector.tensor_scalar_mul(neg_all[:, ft * K:(ft + 1) * K],
                                                ps[:], -1.0)
            rd3 = rd_all[:].rearrange("p (b i) -> p b i", b=N_FT)

            # ---- pairwise-win counting ----
            cntA = cps.tile([128, K], F32)   # j in [0, 128)   -> row j
            cntB = cps.tile([128, K], F32)   # j in [128, 256) -> row j-128
            nc.tensor.matmul(cntA[:], zw[:], zsb[:], start=True, stop=False)
            nc.tensor.matmul(cntB[:], zw[:], zsb[:], start=True, stop=False)

            def cnt_mm(j, rhs):
                jj = j % 128
                c, m = jj // 32, jj % 32
                dst = cntA if j < 128 else cntB
                nc.tensor.matmul(dst[c * 32:(c + 1) * 32, 0:rhs.shape[-1]],
                                 strip[:, 31 - m:63 - m], rhs,
                                 start=False, stop=False, tile_position=(0, c * 32))

            def act_cmp(j, ft):
                L = j + (j & 1)
                ge = gact_pool.tile([128, K], BF16, tag="ge1")
                nc.scalar.activation(
                    ge[:, 0:L], rd_all[:, ft * K:ft * K + L],
                    mybir.ActivationFunctionType.Sign,
                    bias=neg_all[:, ft * K + j:ft * K + j + 1], scale=1.0)
                return ge

            # DVE compares are batched: one tensor_tensor covers a run of
            # consecutive j's x all 4 fruit blocks via a 4-D access pattern
            # (stride-0 broadcasts), amortizing the per-op overhead.
            GE_COLS = 4096                  # bf16 -> 8KB/partition per tile
            groups = []
            j = 1
            while j < T_ACT:
                g = [j]
                while (len(g) < 8 and j + 1 < T_ACT
                       and (len(g) + 1) * N_FT * (j + 1 + ((j + 1) & 1)) <= GE_COLS):
                    j += 1
                    g.append(j)
                groups.append(g)
                j += 1
            act_order = [jj for jj in range(1, 256) if _j_engine(jj) == "act"]

            # Pre-buffer ScalarE compares (blocks 0/1 of the first ACT j's):
            # a (j, ft) compare only needs Rd block ft, so ACT starts while
            # the GEMM is still producing later blocks and stays a few
            # instructions ahead of PE's consumption afterwards.
            hoist = set(act_order[:16])
            act_ge = {}
            for ft in range(2):
                for jj in act_order:
                    if jj in hoist:
                        act_ge[(jj, ft)] = act_cmp(jj, ft)

            # weave ACT js between groups, exhausting them before the last
            # few groups so ACT never paces the kernel tail
            ng = len(groups)
            ai = 0
            for g_idx, grp in enumerate(groups):
                j0 = grp[0]
                G = len(grp)
                Lg = grp[-1] + (grp[-1] & 1)
                ge = gep.tile([128, GE_COLS], BF16, tag="ge")
                in0 = rd3[:, :, 0:Lg].unsqueeze(1).broadcast_to((128, G, N_FT, Lg))
                cols = rd3[:, :, j0:j0 + G].rearrange("p b g -> p g b")
                in1 = cols.unsqueeze(3).broadcast_to((128, G, N_FT, Lg))
                out = ge[:, 0:G * N_FT * Lg].rearrange(
                    "p (g b i) -> p g b i", g=G, b=N_FT)
                nc.vector.tensor_tensor(out, in0, in1, mybir.AluOpType.is_ge)
                for gi, jj in enumerate(grp):
                    Lj = jj + (jj & 1)
                    for b in range(N_FT):
                        cnt_mm(jj, ge[:, (gi * N_FT + b) * Lg:
                                      (gi * N_FT + b) * Lg + Lj])
                # ACT units after this group
                remaining_groups = max(1, ng - 4 - g_idx)
                quota = -(-(len(act_order) - ai) // remaining_groups) \
                    if g_idx < ng - 4 else 0
                for _ in range(quota):
                    if ai >= len(act_order):
                        break
                    jj = act_order[ai]; ai += 1
                    Lj = jj + (jj & 1)
                    for ft in range(N_FT):
                        gea = act_ge.pop((jj, ft), None) or act_cmp(jj, ft)
                        cnt_mm(jj, gea[:, 0:Lj])
            while ai < len(act_order):
                jj = act_order[ai]; ai += 1
                Lj = jj + (jj & 1)
                for ft in range(N_FT):
                    gea = act_ge.pop((jj, ft), None) or act_cmp(jj, ft)
                    cnt_mm(jj, gea[:, 0:Lj])

            # close both accumulation groups across all 128 partitions
            nc.tensor.matmul(cntA[:], zw[:], zsb[:], start=False, stop=True)
            nc.tensor.matmul(cntB[:], zw[:], zsb[:], start=False, stop=True)

            out_sb = pp.tile([128, 2 * K], F32, tag="out")
            nc.scalar.copy(out_sb[:, 0:K], cntA[:])
            nc.scalar.copy(out_sb[:, K:2 * K], cntB[:])
            nc.sync.dma_start(counts[:], out_sb[:])

    nc.compile()
    return nc


def _host_inputs(fruits_prop, tools_prop, M_fruit, M_tool, M, domain_f, domain_t):
    G = np.ascontiguousarray(np.asarray(fruits_prop, np.float32)[np.asarray(domain_f, np.int64)])
    toolsD = np.asarray(tools_prop, np.float32)[np.asarray(domain_t, np.int64)]
    mf = np.ascontiguousarray(np.asarray(M_fruit, np.float32))
    mt = np.asarray(M_tool, np.float32)
    m = np.asarray(M, np.float32)
    WT = np.ascontiguousarray(((toolsD @ mt) @ m).T)   # [512, 256]
    in_maps = []
    for c in range(N_CORES):
        Gc = G[c * F_PER_CORE:(c + 1) * F_PER_CORE]
        in_maps.append({
            "g_t": np.ascontiguousarray(Gc.T),
            "mf": mf,
            "wt_in": WT,
        })
    return in_maps


def decode_counts(counts_sum):
    """counts_sum: [128, 512] summed over cores -> C[i, j] win counts (i < j)."""
    sign_rows = _sign_rows()
    C = np.zeros((K_DOMAIN, K_DOMAIN), np.float64)
    for j in range(1, K_DOMAIN):
        half = j // 128
        row = counts_sum[j % 128, half * K_DOMAIN: half * K_DOMAIN + j]
        if j in sign_rows:
            C[:j, j] = np.rint((row + BATCH) / 2.0)
        else:
            C[:j, j] = np.rint(row)
    return C


def _predict(C, domain_t, tools_labels):
    pos = np.full(N_TOOLS, -1, np.int64)
    pos[np.asarray(domain_t, np.int64)] = np.arange(K_DOMAIN)
    l1 = pos[np.asarray(tools_labels[0], np.int64)]
    l2 = pos[np.asarray(tools_labels[1], np.int64)]
    B = l1.shape[0]
    half = BATCH // 2
    choice = np.ones(B, np.int64)
    lt = (l1 >= 0) & (l2 >= 0) & (l1 < l2)
    gt = (l1 >= 0) & (l2 >= 0) & (l1 > l2)
    choice[lt] = np.where(C[l1[lt], l2[lt]] >= half, 0, 1)
    choice[gt] = np.where(C[l2[gt], l1[gt]] <= half, 0, 1)
    out = np.zeros((B, 4), np.float32)
    out[:, 0] = 1.0
    out[np.arange(B), 1 + choice] = 1.0
    return out


def kernel(fruits_prop, tools_prop, M_fruit, M_tool, M, min_r, domain_f,
           domain_t, tools_labels):
    global LAST_RESULTS
    in_maps = _host_inputs(fruits_prop, tools_prop, M_fruit, M_tool, M,
                           domain_f, domain_t)
    nc = _build_nc()
    res = run_bass_kernel_spmd(nc, in_maps, list(range(N_CORES)), trace=TRACE)
    LAST_RESULTS = res
    counts_sum = np.zeros((128, 2 * K_DOMAIN), np.float64)
    for c in range(N_CORES):
        counts_sum += res.results[c]["counts"].astype(np.float64)
    C = decode_counts(counts_sum)
    return _predict(C, domain_t, tools_labels)
